# revision 22
# baseline (speedup 1.0000x reference)
"""BinaryConv2d (3x3, stride 1, pad 1) on 8 Trainium2 NeuronCores.

Data-parallel over batch: 32 images -> 4 per core, weights replicated.

The 2e-2 correctness gate comfortably admits fp16 I/O staging, so all HBM
traffic is fp16 (halving the memory-roofline time vs fp32):

- Host prep: the input is zero-padded to [4, 64, 114, 114] and cast to fp16;
  the binarized weight sign(w)*alpha (alpha folded in, per out-channel) goes
  to fp16 lhsT layout [c, tap*K + k]. Device-side preprocessing is zero: the
  padded plane DMAs straight into SBUF, fully contiguous per partition.
- Device out: fp16 in a permuted layout [img, hh, k, ss, r, w] so each
  output DMA is one 128-partition transfer with long contiguous DRAM runs;
  the host transposes back to NCHW and upcasts to fp32.

Per-core kernel: images are processed in pairs. The pair's 2x64 input
channels fill the 128 SBUF partitions, each holding a zero-padded 114x114
fp16 image plane. The 3x3 conv is 9 PSUM-accumulated matmuls per 4-row
output chunk: lhsT = [c, k] tap weights, rhs = the padded plane shifted by
the tap offset (pure AP arithmetic). Four matmul streams run concurrently on
the four 64x64 PE array quadrants: (image A, image B) x (chunk c, chunk
c+1). PSUM->SBUF eviction (fp32->fp16 cast) alternates ScalarE/VectorE.

Measured bottleneck (via do_mm/do_evict/do_outdma probes): the kernel is
PE *instruction-stream* bound — every matmul lowers to LDWEIGHTS+MATMUL
(~66 ns serial engine cost: ~53 ns 64-col weight load + issue), capping the
4-quadrant stream concurrency at ~3.1. Evictions and both DMA directions
are fully hidden under the matmul stream. Alternatives measured slower:
tap-outer ("wave") order 81.6 us, plane-linear 512-wide chunks (900 vs 1008
matmuls but longer streams) 71.6 us, SPREAD order 71.5 us, vs 68.5-70 us
for this config; fp32 I/O baseline was 93.9 us.
"""

import numpy as np

import concourse.bass as bass
import concourse.tile as tile
from concourse import bacc, mybir
from concourse.bass_utils import run_bass_kernel_spmd

N_CORES = 8
N_PER_CORE = 4  # images per core (batch 32 / 8 cores)
C = 64          # input channels
K = 64          # output channels
H = W = 112
HP, WP = H + 2, W + 2   # zero-padded plane (padded on host)
R = 4                   # output rows per PSUM half-chunk (R*W = 448 <= 512)
NSUPER = H // (2 * R)   # 14 superchunks (8 rows each) per image pair
BANDS = [12, 26, 26, 26, 24]          # padded-row input DMA bands (sum 114)
OGROUPS = [(0, 7), (7, 4), (11, 2), (13, 1)]  # (start, len) superchunk groups
F16 = mybir.dt.float16
F32 = mybir.dt.float32

# plane-linear scheme ("lin"): the matmul free dim is a contiguous
# 512-element window of the padded plane (tap shift = pure offset). The two
# pad columns per row compute garbage that the host drops. 25 chunks of 512
# cover the H*WP = 12768 output positions per image (vs 28 448-wide chunks),
# cutting matmul+LDWEIGHTS count 1008 -> 900 per core; LDWEIGHTS issue is
# the serial bottleneck on the PE weight path.
FLIN = H * WP            # 12768 plane-linear output extent
NCHUNK = 25              # ceil(12768 / 512)
NCP = 13                 # chunk pairs per image (even chunk 24 is lone)
SLACK = 256              # xpad tail slack: garbage outputs read past plane
LIN_OGROUPS = [(0, 7), (7, 6)]        # (start, len) chunk-pair DMA groups


ORDERS = {
    "AABB": [(0, 0), (0, 1), (1, 0), (1, 1)],    # img outer, half inner
    "ABAB": [(0, 0), (1, 0), (0, 1), (1, 1)],    # half outer, img inner
    "SPREAD": [(0, 0), (1, 1), (0, 1), (1, 0)],  # no shared row/col adjacency
}


def _build_nc(dyn_rep=False, do_mm=True, do_evict=True, do_outdma=True,
              wave=False, lin=False, order="AABB", streams=None, thin=0):
    """Build the per-core program. dyn_rep=True adds a "rep" [1,1] int32
    input and wraps the body in a hardware For_i loop with that runtime trip
    count (timing only; the computation is idempotent). do_mm/do_evict/
    do_outdma gate kernel stages for timing probes (all True for real use).
    lin=False (default) uses the 4-row 448-wide chunk scheme (fastest
    measured); lin=True the plane-linear 512-wide chunk scheme. wave=True
    (only with lin=False) reorders matmuls tap-outer within each OGROUP
    wave (slower; kept for reference)."""
    nc = bacc.Bacc(
        "TRN2", target_bir_lowering=False, debug=False, num_devices=N_CORES
    )
    x_d = nc.dram_tensor("x", [N_PER_CORE, C, HP, WP], F16, kind="ExternalInput")
    wt_d = nc.dram_tensor("wt", [128, 9 * K], F16, kind="ExternalInput")
    if dyn_rep:
        rep_d = nc.dram_tensor("rep", [1, 1], mybir.dt.int32, kind="ExternalInput")
    if lin:
        out_d = nc.dram_tensor(
            "out", [N_PER_CORE, 128, NCP * 512], F16, kind="ExternalOutput"
        )
    else:
        out_d = nc.dram_tensor(
            "out", [N_PER_CORE, 2, K, NSUPER, R, W], F16, kind="ExternalOutput"
        )

    from contextlib import nullcontext

    with tile.TileContext(nc) as tc:
        rep_ctx = nullcontext()
        if dyn_rep:
            with tc.tile_pool(name="reppool", bufs=1) as reppool:
                rep_sb = reppool.tile([1, 1], mybir.dt.int32)
                nc.sync.dma_start(out=rep_sb[:], in_=rep_d[:])
                rv = nc.values_load(rep_sb[0:1, 0:1])
            rep_ctx = tc.For_i(
                0, rv, 1,
                hint_engines=(mybir.EngineType.PE, mybir.EngineType.SP,
                              mybir.EngineType.DVE, mybir.EngineType.Activation),
            )
        with (
            tc.tile_pool(name="wpool", bufs=1) as wpool,
            tc.tile_pool(name="xpool", bufs=2) as xpool,
            tc.tile_pool(name="opool", bufs=2) as opool,
            tc.tile_pool(name="pspool", bufs=8, space="PSUM") as pspool,
            rep_ctx,
        ):
            w_sb = wpool.tile([128, 9 * K], F16)
            nc.sync.dma_start(out=w_sb[:], in_=wt_d[:])

            for pair in range(N_PER_CORE // 2):
                xpad = xpool.tile([128, HP * WP + (SLACK if lin else 0)], F16)
                if lin:
                    nc.vector.memset(xpad[:, HP * WP :], 0.0)
                r0 = 0
                for brows in BANDS:
                    nc.sync.dma_start(
                        out=xpad[:, r0 * WP : (r0 + brows) * WP],
                        in_=x_d[2 * pair : 2 * pair + 2, :, r0 : r0 + brows, :]
                        .rearrange("n c h w -> (n c) (h w)"),
                    )
                    r0 += brows

                if lin:
                    osb = [
                        opool.tile([128, NCP * 512], F16,
                                   name=f"osb{i}", tag=f"osb{i}")
                        for i in range(2)
                    ]
                    for cp in range(NCP):
                        psa = pspool.tile([128, 512], F32, name="psa", tag="ps")
                        psb = pspool.tile([128, 512], F32, name="psb", tag="ps")
                        pars = (0, 1) if cp < NCP - 1 else (0,)
                        if do_mm:
                            for t in range(9):
                                dy, dx = divmod(t, 3)
                                delta = dy * WP + dx
                                for img, par in ORDERS[order]:
                                    if par not in pars:
                                        continue
                                    p0 = img * 64
                                    ps = (psa, psb)[img]
                                    base = (2 * cp + par) * 512 + delta
                                    nc.tensor.matmul(
                                        ps[par * 64 : (par + 1) * 64, :],
                                        w_sb[p0 : p0 + 64, t * K : (t + 1) * K],
                                        xpad[p0 : p0 + 64, base : base + 512],
                                        start=(t == 0),
                                        stop=(t == 8),
                                        skip_group_check=True,
                                    )
                        if do_evict:
                            for img in (0, 1):
                                ps = (psa, psb)[img]
                                if cp < NCP - 1:
                                    dst = osb[img][:, cp * 512 : (cp + 1) * 512]
                                    src = ps[:]
                                else:  # lone even chunk: partitions 0-63 only
                                    dst = osb[img][0:64, cp * 512 : (cp + 1) * 512]
                                    src = ps[0:64, :]
                                if (cp + img) % 2 == 0:
                                    nc.scalar.copy(dst, src)
                                else:
                                    nc.vector.tensor_copy(dst, src)
                    if do_outdma:
                        for g0, glen in LIN_OGROUPS:
                            for img in (0, 1):
                                nc.sync.dma_start(
                                    out=out_d[2 * pair + img]
                                    [:, g0 * 512 : (g0 + glen) * 512],
                                    in_=osb[img][:, g0 * 512 : (g0 + glen) * 512],
                                )
                    continue

                v = xpad.rearrange("p (h w) -> p h w", h=HP)

                groups = [(0, 4), (4, 4), (8, 4), (12, 2)] if wave else OGROUPS
                for g0, glen in groups:
                    ost = [
                        opool.tile([128, glen * R * W], F16,
                                   name=f"ost{i}", tag=f"ost{i}")
                        for i in range(2)
                    ]
                    if wave:
                        # tap-outer order: the glen matmuls per (ch, img, t)
                        # share lhsT (one LDWEIGHTS amortized glen ways) and
                        # consecutive quadrant blocks alternate PE row groups
                        # so every LDWEIGHTS can pull ahead of in-flight MMs.
                        pss = [
                            [pspool.tile([128, R * W], F32, name="ps", tag="ps")
                             for img in (0, 1)]
                            for s in range(glen)
                        ]
                        if do_mm:
                            for t in range(9):
                                dy, dx = divmod(t, 3)
                                for ch in (0, 1):
                                    for img in (0, 1):
                                        p0 = img * 64
                                        for s in range(glen):
                                            yy = (g0 + s) * 2 * R + ch * R + dy
                                            nc.tensor.matmul(
                                                pss[s][img][ch * 64 : ch * 64 + 64, :],
                                                w_sb[p0 : p0 + 64, t * K : (t + 1) * K],
                                                v[p0 : p0 + 64, yy : yy + R, dx : dx + W],
                                                start=(t == 0),
                                                stop=(t == 8),
                                                skip_group_check=True,
                                            )
                        if do_evict:
                            for s in range(glen):
                                for img in (0, 1):
                                    dst = ost[img][:, s * R * W : (s + 1) * R * W]
                                    if (s + img) % 2 == 0:
                                        nc.scalar.copy(dst, pss[s][img][:])
                                    else:
                                        nc.vector.tensor_copy(dst, pss[s][img][:])
                    else:
                        for s in range(glen):
                            y0 = (g0 + s) * 2 * R
                            psa = pspool.tile([128, R * W], F32, name="psa", tag="ps")
                            psb = pspool.tile([128, R * W], F32, name="psb", tag="ps")
                            if do_mm:
                                for t in range(9):
                                    dy, dx = divmod(t, 3)
                                    for img, ch in (streams or ORDERS[order]):
                                        p0 = img * 64
                                        ps = (psa, psb)[img]
                                        yy = y0 + ch * R + dy
                                        if thin:  # timing probe: free dim cut
                                            nc.tensor.matmul(
                                                ps[ch * 64 : ch * 64 + 64, 0:thin],
                                                w_sb[p0 : p0 + 64, t * K : (t + 1) * K],
                                                v[p0 : p0 + 64, yy, dx : dx + thin],
                                                start=(t == 0),
                                                stop=(t == 8),
                                                skip_group_check=True,
                                            )
                                            continue
                                        nc.tensor.matmul(
                                            ps[ch * 64 : ch * 64 + 64, :],
                                            w_sb[p0 : p0 + 64, t * K : (t + 1) * K],
                                            v[p0 : p0 + 64, yy : yy + R, dx : dx + W],
                                            start=(t == 0),
                                            stop=(t == 8),
                                            skip_group_check=True,
                                        )
                            if do_evict:
                                for img in (0, 1):
                                    dst = ost[img][:, s * R * W : (s + 1) * R * W]
                                    src = (psa, psb)[img][:]
                                    if (s + img) % 2 == 0:
                                        nc.scalar.copy(dst, src)
                                    else:
                                        nc.vector.tensor_copy(dst, src)
                    if do_outdma:
                        for img in (0, 1):
                            nc.sync.dma_start(
                                out=out_d[2 * pair + img][:, :, g0 : g0 + glen]
                                .rearrange("hh c ss r w -> (hh c) (ss r w)"),
                                in_=ost[img][:],
                            )
    nc.compile()
    return nc


_NC_CACHE = None


def _get_nc():
    global _NC_CACHE
    if _NC_CACHE is None:
        _NC_CACHE = _build_nc()
    return _NC_CACHE


def _prep_weight(weight, alpha):
    weight = np.asarray(weight, dtype=np.float32)
    alpha = np.asarray(alpha, dtype=np.float32).reshape(K, 1, 1)
    sgn = np.where(weight >= 0, np.float32(1.0), np.float32(-1.0))
    bw = (sgn.reshape(K, C, 9) * alpha).astype(np.float16)     # [k, c, t]
    arr = bw.transpose(1, 2, 0).reshape(C, 9 * K)              # [c, t*K + k]
    return np.ascontiguousarray(np.concatenate([arr, arr], axis=0))  # [128, 9K]


def _prep_input(input):
    x = np.asarray(input)
    n = x.shape[0]
    xp = np.zeros((n, C, HP, WP), np.float16)
    xp[:, :, 1 : H + 1, 1 : W + 1] = x
    return xp


def _unpack_out(full):
    if full.ndim == 3:
        # lin: [n, (par k), (cp q)] -> [n, k, h, w]; f = (2*cp + par)*512 + q
        # is the plane-linear position (y, x) = divmod(f, WP), x >= W dropped.
        arr = full.reshape(-1, 2, K, NCP, 512).transpose(0, 2, 3, 1, 4)
        arr = arr.reshape(-1, K, NCP * 2 * 512)[:, :, :FLIN]
        arr = arr.reshape(-1, K, H, WP)[:, :, :, :W]
        return np.ascontiguousarray(arr).astype(np.float32)
    # 448 scheme: [n, hh, c, ss, r, w] -> [n, c, (ss hh r), w]
    out = np.ascontiguousarray(full.transpose(0, 2, 3, 1, 4, 5))
    return out.reshape(-1, K, H, W).astype(np.float32)


def run_sharded(inputs, trace=False, **kw):
    xp = _prep_input(inputs["input"])
    wt = _prep_weight(inputs["weight"], inputs["alpha"])
    nc = _get_nc()
    in_maps = [
        {"x": xp[i * N_PER_CORE : (i + 1) * N_PER_CORE], "wt": wt}
        for i in range(N_CORES)
    ]
    res = run_bass_kernel_spmd(nc, in_maps, list(range(N_CORES)), trace=trace, **kw)
    full = np.concatenate(
        [res.results[i]["out"] for i in range(N_CORES)], axis=0
    )
    return _unpack_out(full), res


def kernel(**inputs) -> np.ndarray:
    out, _ = run_sharded(inputs)
    return out


def _timed_runner(nc, inputs, extra=None):
    """Build a jitted 8-core runner for `nc` and device-resident args."""
    import jax
    from jax.experimental.shard_map import shard_map
    from jax.sharding import Mesh, NamedSharding, PartitionSpec

    from concourse import bass2jax

    bass2jax.install_neuronx_cc_hook()
    xp = _prep_input(inputs["input"])
    wt = _prep_weight(inputs["weight"], inputs["alpha"])

    partition_name = nc.partition_id_tensor.name if nc.partition_id_tensor else None
    in_names, out_names, out_avals, zero_outs = [], [], [], []
    for alloc in nc.m.functions[0].allocations:
        if not isinstance(alloc, mybir.MemoryLocationSet):
            continue
        name = alloc.memorylocations[0].name
        if alloc.kind == "ExternalInput":
            if name != partition_name:
                in_names.append(name)
        elif alloc.kind == "ExternalOutput":
            shape = tuple(alloc.tensor_shape)
            dtype = mybir.dt.np(alloc.dtype)
            out_names.append(name)
            out_avals.append(jax.core.ShapedArray(shape, dtype))
            zero_outs.append(np.zeros(shape, dtype))
    n_params = len(in_names)

    def _body(*args):
        operands = list(args)
        if partition_name is not None:
            operands.append(bass2jax.partition_id_tensor())
        outs = bass2jax._bass_exec_p.bind(
            *operands,
            out_avals=tuple(out_avals),
            in_names=tuple(
                in_names + out_names + ([partition_name] if partition_name else [])
            ),
            out_names=tuple(out_names),
            lowering_input_output_aliases=(),
            sim_require_finite=True,
            sim_require_nnan=True,
            nc=nc,
        )
        return tuple(outs)

    devices = jax.devices()[:N_CORES]
    mesh = Mesh(np.asarray(devices), ("core",))
    spec = PartitionSpec("core")
    nshard = NamedSharding(mesh, spec)
    fn = jax.jit(
        shard_map(
            _body,
            mesh=mesh,
            in_specs=(spec,) * (n_params + len(out_names)),
            out_specs=(spec,) * len(out_names),
            check_rep=False,
        ),
        keep_unused=True,
    )
    per_core = {
        "x": [xp[i * N_PER_CORE : (i + 1) * N_PER_CORE] for i in range(N_CORES)],
        "wt": [wt] * N_CORES,
    }
    for name, arr in (extra or {}).items():
        per_core[name] = [arr] * N_CORES
    args = [np.concatenate(per_core[name], axis=0) for name in in_names] + [
        np.zeros((N_CORES * z.shape[0], *z.shape[1:]), z.dtype) for z in zero_outs
    ]
    dev_args = [jax.device_put(a, nshard) for a in args]
    idx = {name: i for i, name in enumerate(in_names)}
    return fn, dev_args, idx, nshard


def time_kernel(inputs, rep_big=1025, pairs=8, **build_kw):
    """Isolate on-device kernel time with ONE executable whose For_i trip
    count is a runtime input: wall(rep_big) - wall(1), / (rep_big - 1).
    Alternates the two trip counts to cancel slow drift."""
    import time

    import jax

    nc = _build_nc(dyn_rep=True, **build_kw)
    fn, dev_args, idx, nshard = _timed_runner(
        nc, inputs, extra={"rep": np.array([[1]], np.int32)}
    )
    ri = idx["rep"]

    def arg_set(k):
        a = list(dev_args)
        a[ri] = jax.device_put(
            np.concatenate([np.array([[k]], np.int32)] * N_CORES, axis=0), nshard
        )
        return a

    a1, ab = arg_set(1), arg_set(rep_big)
    for a in (a1, ab):  # compile + warm both trip counts
        jax.block_until_ready(fn(*a))

    t1s, tbs = [], []
    for _ in range(pairs):
        t0 = time.perf_counter()
        jax.block_until_ready(fn(*a1))
        t1s.append(time.perf_counter() - t0)
        t0 = time.perf_counter()
        jax.block_until_ready(fn(*ab))
        tbs.append(time.perf_counter() - t0)
    t1, tb = min(t1s), min(tbs)
    per_exec = (tb - t1) / (rep_big - 1)
    return per_exec * 1e9, {"t1": t1s, "tbig": tbs, "rep_big": rep_big}


# revision 28
# speedup vs baseline: 1.0496x; 1.0496x over previous
"""BinaryConv2d (3x3, stride 1, pad 1) on 8 Trainium2 NeuronCores.

Data-parallel over batch: 32 images -> 4 per core, weights replicated.

The 2e-2 correctness gate comfortably admits fp16 I/O staging, so all HBM
traffic is fp16 (halving the memory-roofline time vs fp32):

- Host prep: the input is zero-padded to [4, 64, 114, 114] and cast to fp16;
  the binarized weight sign(w)*alpha (alpha folded in, per out-channel) goes
  to fp16 lhsT layout [c, tap*K + k]. Device-side preprocessing is zero: the
  padded plane DMAs straight into SBUF, fully contiguous per partition.
- Device out: fp16 in a permuted layout [img, hh, k, ss, r, w] so each
  output DMA is one 128-partition transfer with long contiguous DRAM runs;
  the host transposes back to NCHW and upcasts to fp32.

Per-core kernel: images are processed in pairs. The pair's 2x64 input
channels fill the 128 SBUF partitions, each holding a zero-padded 114x114
fp16 image plane. The 3x3 conv is 9 PSUM-accumulated matmuls per 4-row
output chunk: lhsT = [c, k] tap weights, rhs = the padded plane shifted by
the tap offset (pure AP arithmetic). Four matmul streams run concurrently on
the four 64x64 PE array quadrants: (image A, image B) x (chunk c, chunk
c+1). PSUM->SBUF eviction (fp32->fp16 cast) runs on ScalarE ("trim"
config: keeping VectorE out of the timing For_i loop and using 2 output-DMA
groups measured ~1-2 us faster than alternating engines + 4 groups).

Measured bottleneck (via do_mm/do_evict/do_outdma probes): the kernel is
PE *instruction-stream* bound — every matmul lowers to LDWEIGHTS+MATMUL
(~66 ns serial engine cost: ~53 ns 64-col weight load + issue), capping the
4-quadrant stream concurrency at ~3.1. Evictions and both DMA directions
are fully hidden under the matmul stream. Alternatives measured slower:
tap-outer ("wave") order 81.6 us, plane-linear 512-wide chunks (900 vs 1008
matmuls but longer streams) 71.6 us, SPREAD order 71.5 us, vs 68.5-70 us
for this config; fp32 I/O baseline was 93.9 us.
"""

import numpy as np

import concourse.bass as bass
import concourse.tile as tile
from concourse import bacc, mybir
from concourse.bass_utils import run_bass_kernel_spmd

N_CORES = 8
N_PER_CORE = 4  # images per core (batch 32 / 8 cores)
C = 64          # input channels
K = 64          # output channels
H = W = 112
HP, WP = H + 2, W + 2   # zero-padded plane (padded on host)
R = 4                   # output rows per PSUM half-chunk (R*W = 448 <= 512)
NSUPER = H // (2 * R)   # 14 superchunks (8 rows each) per image pair
BANDS = [12, 26, 26, 26, 24]          # padded-row input DMA bands (sum 114)
OGROUPS = [(0, 7), (7, 4), (11, 2), (13, 1)]  # (start, len) superchunk groups
F16 = mybir.dt.float16
F32 = mybir.dt.float32

# plane-linear scheme ("lin"): the matmul free dim is a contiguous
# 512-element window of the padded plane (tap shift = pure offset). The two
# pad columns per row compute garbage that the host drops. 25 chunks of 512
# cover the H*WP = 12768 output positions per image (vs 28 448-wide chunks),
# cutting matmul+LDWEIGHTS count 1008 -> 900 per core; LDWEIGHTS issue is
# the serial bottleneck on the PE weight path.
FLIN = H * WP            # 12768 plane-linear output extent
NCHUNK = 25              # ceil(12768 / 512)
NCP = 13                 # chunk pairs per image (even chunk 24 is lone)
SLACK = 256              # xpad tail slack: garbage outputs read past plane
LIN_OGROUPS = [(0, 7), (7, 6)]        # (start, len) chunk-pair DMA groups


ORDERS = {
    "AABB": [(0, 0), (0, 1), (1, 0), (1, 1)],    # img outer, half inner
    "ABAB": [(0, 0), (1, 0), (0, 1), (1, 1)],    # half outer, img inner
    "SPREAD": [(0, 0), (1, 1), (0, 1), (1, 0)],  # no shared row/col adjacency
}


def _build_nc(dyn_rep=False, do_mm=True, do_evict=True, do_outdma=True,
              wave=False, lin=False, order="AABB", streams=None, thin=0,
              trim=True):
    """Build the per-core program. dyn_rep=True adds a "rep" [1,1] int32
    input and wraps the body in a hardware For_i loop with that runtime trip
    count (timing only; the computation is idempotent). do_mm/do_evict/
    do_outdma gate kernel stages for timing probes (all True for real use).
    lin=False (default) uses the 4-row 448-wide chunk scheme (fastest
    measured); lin=True the plane-linear 512-wide chunk scheme. wave=True
    (only with lin=False) reorders matmuls tap-outer within each OGROUP
    wave (slower; kept for reference)."""
    nc = bacc.Bacc(
        "TRN2", target_bir_lowering=False, debug=False, num_devices=N_CORES
    )
    x_d = nc.dram_tensor("x", [N_PER_CORE, C, HP, WP], F16, kind="ExternalInput")
    wt_d = nc.dram_tensor("wt", [128, 9 * K], F16, kind="ExternalInput")
    if dyn_rep:
        rep_d = nc.dram_tensor("rep", [1, 1], mybir.dt.int32, kind="ExternalInput")
    if lin:
        out_d = nc.dram_tensor(
            "out", [N_PER_CORE, 128, NCP * 512], F16, kind="ExternalOutput"
        )
    else:
        out_d = nc.dram_tensor(
            "out", [N_PER_CORE, 2, K, NSUPER, R, W], F16, kind="ExternalOutput"
        )

    from contextlib import nullcontext

    with tile.TileContext(nc) as tc:
        rep_ctx = nullcontext()
        if dyn_rep:
            with tc.tile_pool(name="reppool", bufs=1) as reppool:
                rep_sb = reppool.tile([1, 1], mybir.dt.int32)
                nc.sync.dma_start(out=rep_sb[:], in_=rep_d[:])
                rv = nc.values_load(rep_sb[0:1, 0:1])
            engines = ((mybir.EngineType.PE, mybir.EngineType.SP,
                        mybir.EngineType.Activation) if trim else
                       (mybir.EngineType.PE, mybir.EngineType.SP,
                        mybir.EngineType.DVE, mybir.EngineType.Activation))
            rep_ctx = tc.For_i(0, rv, 1, hint_engines=engines)
        with (
            tc.tile_pool(name="wpool", bufs=1) as wpool,
            tc.tile_pool(name="xpool", bufs=2) as xpool,
            tc.tile_pool(name="opool", bufs=2) as opool,
            tc.tile_pool(name="pspool", bufs=8, space="PSUM") as pspool,
            rep_ctx,
        ):
            w_sb = wpool.tile([128, 9 * K], F16)
            nc.sync.dma_start(out=w_sb[:], in_=wt_d[:])

            for pair in range(N_PER_CORE // 2):
                xpad = xpool.tile([128, HP * WP + (SLACK if lin else 0)], F16)
                if lin:
                    nc.vector.memset(xpad[:, HP * WP :], 0.0)
                r0 = 0
                for brows in BANDS:
                    nc.sync.dma_start(
                        out=xpad[:, r0 * WP : (r0 + brows) * WP],
                        in_=x_d[2 * pair : 2 * pair + 2, :, r0 : r0 + brows, :]
                        .rearrange("n c h w -> (n c) (h w)"),
                    )
                    r0 += brows

                if lin:
                    osb = [
                        opool.tile([128, NCP * 512], F16,
                                   name=f"osb{i}", tag=f"osb{i}")
                        for i in range(2)
                    ]
                    for cp in range(NCP):
                        psa = pspool.tile([128, 512], F32, name="psa", tag="ps")
                        psb = pspool.tile([128, 512], F32, name="psb", tag="ps")
                        pars = (0, 1) if cp < NCP - 1 else (0,)
                        if do_mm:
                            for t in range(9):
                                dy, dx = divmod(t, 3)
                                delta = dy * WP + dx
                                for img, par in ORDERS[order]:
                                    if par not in pars:
                                        continue
                                    p0 = img * 64
                                    ps = (psa, psb)[img]
                                    base = (2 * cp + par) * 512 + delta
                                    nc.tensor.matmul(
                                        ps[par * 64 : (par + 1) * 64, :],
                                        w_sb[p0 : p0 + 64, t * K : (t + 1) * K],
                                        xpad[p0 : p0 + 64, base : base + 512],
                                        start=(t == 0),
                                        stop=(t == 8),
                                        skip_group_check=True,
                                    )
                        if do_evict:
                            for img in (0, 1):
                                ps = (psa, psb)[img]
                                if cp < NCP - 1:
                                    dst = osb[img][:, cp * 512 : (cp + 1) * 512]
                                    src = ps[:]
                                else:  # lone even chunk: partitions 0-63 only
                                    dst = osb[img][0:64, cp * 512 : (cp + 1) * 512]
                                    src = ps[0:64, :]
                                if (cp + img) % 2 == 0:
                                    nc.scalar.copy(dst, src)
                                else:
                                    nc.vector.tensor_copy(dst, src)
                    if do_outdma:
                        for g0, glen in LIN_OGROUPS:
                            for img in (0, 1):
                                nc.sync.dma_start(
                                    out=out_d[2 * pair + img]
                                    [:, g0 * 512 : (g0 + glen) * 512],
                                    in_=osb[img][:, g0 * 512 : (g0 + glen) * 512],
                                )
                    continue

                v = xpad.rearrange("p (h w) -> p h w", h=HP)

                groups = ([(0, 4), (4, 4), (8, 4), (12, 2)] if wave
                          else [(0, 7), (7, 7)] if trim else OGROUPS)
                for g0, glen in groups:
                    ost = [
                        opool.tile([128, glen * R * W], F16,
                                   name=f"ost{i}", tag=f"ost{i}")
                        for i in range(2)
                    ]
                    if wave:
                        # tap-outer order: the glen matmuls per (ch, img, t)
                        # share lhsT (one LDWEIGHTS amortized glen ways) and
                        # consecutive quadrant blocks alternate PE row groups
                        # so every LDWEIGHTS can pull ahead of in-flight MMs.
                        pss = [
                            [pspool.tile([128, R * W], F32, name="ps", tag="ps")
                             for img in (0, 1)]
                            for s in range(glen)
                        ]
                        if do_mm:
                            for t in range(9):
                                dy, dx = divmod(t, 3)
                                for ch in (0, 1):
                                    for img in (0, 1):
                                        p0 = img * 64
                                        for s in range(glen):
                                            yy = (g0 + s) * 2 * R + ch * R + dy
                                            nc.tensor.matmul(
                                                pss[s][img][ch * 64 : ch * 64 + 64, :],
                                                w_sb[p0 : p0 + 64, t * K : (t + 1) * K],
                                                v[p0 : p0 + 64, yy : yy + R, dx : dx + W],
                                                start=(t == 0),
                                                stop=(t == 8),
                                                skip_group_check=True,
                                            )
                        if do_evict:
                            for s in range(glen):
                                for img in (0, 1):
                                    dst = ost[img][:, s * R * W : (s + 1) * R * W]
                                    if (s + img) % 2 == 0:
                                        nc.scalar.copy(dst, pss[s][img][:])
                                    else:
                                        nc.vector.tensor_copy(dst, pss[s][img][:])
                    else:
                        for s in range(glen):
                            y0 = (g0 + s) * 2 * R
                            psa = pspool.tile([128, R * W], F32, name="psa", tag="ps")
                            psb = pspool.tile([128, R * W], F32, name="psb", tag="ps")
                            if do_mm:
                                for t in range(9):
                                    dy, dx = divmod(t, 3)
                                    for img, ch in (streams or ORDERS[order]):
                                        p0 = img * 64
                                        ps = (psa, psb)[img]
                                        yy = y0 + ch * R + dy
                                        if thin:  # timing probe: free dim cut
                                            nc.tensor.matmul(
                                                ps[ch * 64 : ch * 64 + 64, 0:thin],
                                                w_sb[p0 : p0 + 64, t * K : (t + 1) * K],
                                                v[p0 : p0 + 64, yy, dx : dx + thin],
                                                start=(t == 0),
                                                stop=(t == 8),
                                                skip_group_check=True,
                                            )
                                            continue
                                        nc.tensor.matmul(
                                            ps[ch * 64 : ch * 64 + 64, :],
                                            w_sb[p0 : p0 + 64, t * K : (t + 1) * K],
                                            v[p0 : p0 + 64, yy : yy + R, dx : dx + W],
                                            start=(t == 0),
                                            stop=(t == 8),
                                            skip_group_check=True,
                                        )
                            if do_evict:
                                for img in (0, 1):
                                    dst = ost[img][:, s * R * W : (s + 1) * R * W]
                                    src = (psa, psb)[img][:]
                                    if trim or (s + img) % 2 == 0:
                                        nc.scalar.copy(dst, src)
                                    else:
                                        nc.vector.tensor_copy(dst, src)
                    if do_outdma:
                        for img in (0, 1):
                            nc.sync.dma_start(
                                out=out_d[2 * pair + img][:, :, g0 : g0 + glen]
                                .rearrange("hh c ss r w -> (hh c) (ss r w)"),
                                in_=ost[img][:],
                            )
    nc.compile()
    return nc


_NC_CACHE = None


def _get_nc():
    global _NC_CACHE
    if _NC_CACHE is None:
        _NC_CACHE = _build_nc()
    return _NC_CACHE


def _prep_weight(weight, alpha):
    weight = np.asarray(weight, dtype=np.float32)
    alpha = np.asarray(alpha, dtype=np.float32).reshape(K, 1, 1)
    sgn = np.where(weight >= 0, np.float32(1.0), np.float32(-1.0))
    bw = (sgn.reshape(K, C, 9) * alpha).astype(np.float16)     # [k, c, t]
    arr = bw.transpose(1, 2, 0).reshape(C, 9 * K)              # [c, t*K + k]
    return np.ascontiguousarray(np.concatenate([arr, arr], axis=0))  # [128, 9K]


def _prep_input(input):
    x = np.asarray(input)
    n = x.shape[0]
    xp = np.zeros((n, C, HP, WP), np.float16)
    xp[:, :, 1 : H + 1, 1 : W + 1] = x
    return xp


def _unpack_out(full):
    if full.ndim == 3:
        # lin: [n, (par k), (cp q)] -> [n, k, h, w]; f = (2*cp + par)*512 + q
        # is the plane-linear position (y, x) = divmod(f, WP), x >= W dropped.
        arr = full.reshape(-1, 2, K, NCP, 512).transpose(0, 2, 3, 1, 4)
        arr = arr.reshape(-1, K, NCP * 2 * 512)[:, :, :FLIN]
        arr = arr.reshape(-1, K, H, WP)[:, :, :, :W]
        return np.ascontiguousarray(arr).astype(np.float32)
    # 448 scheme: [n, hh, c, ss, r, w] -> [n, c, (ss hh r), w]
    out = np.ascontiguousarray(full.transpose(0, 2, 3, 1, 4, 5))
    return out.reshape(-1, K, H, W).astype(np.float32)


def run_sharded(inputs, trace=False, **kw):
    xp = _prep_input(inputs["input"])
    wt = _prep_weight(inputs["weight"], inputs["alpha"])
    nc = _get_nc()
    in_maps = [
        {"x": xp[i * N_PER_CORE : (i + 1) * N_PER_CORE], "wt": wt}
        for i in range(N_CORES)
    ]
    res = run_bass_kernel_spmd(nc, in_maps, list(range(N_CORES)), trace=trace, **kw)
    full = np.concatenate(
        [res.results[i]["out"] for i in range(N_CORES)], axis=0
    )
    return _unpack_out(full), res


def kernel(**inputs) -> np.ndarray:
    out, _ = run_sharded(inputs)
    return out


def _timed_runner(nc, inputs, extra=None):
    """Build a jitted 8-core runner for `nc` and device-resident args."""
    import jax
    from jax.experimental.shard_map import shard_map
    from jax.sharding import Mesh, NamedSharding, PartitionSpec

    from concourse import bass2jax

    bass2jax.install_neuronx_cc_hook()
    xp = _prep_input(inputs["input"])
    wt = _prep_weight(inputs["weight"], inputs["alpha"])

    partition_name = nc.partition_id_tensor.name if nc.partition_id_tensor else None
    in_names, out_names, out_avals, zero_outs = [], [], [], []
    for alloc in nc.m.functions[0].allocations:
        if not isinstance(alloc, mybir.MemoryLocationSet):
            continue
        name = alloc.memorylocations[0].name
        if alloc.kind == "ExternalInput":
            if name != partition_name:
                in_names.append(name)
        elif alloc.kind == "ExternalOutput":
            shape = tuple(alloc.tensor_shape)
            dtype = mybir.dt.np(alloc.dtype)
            out_names.append(name)
            out_avals.append(jax.core.ShapedArray(shape, dtype))
            zero_outs.append(np.zeros(shape, dtype))
    n_params = len(in_names)

    def _body(*args):
        operands = list(args)
        if partition_name is not None:
            operands.append(bass2jax.partition_id_tensor())
        outs = bass2jax._bass_exec_p.bind(
            *operands,
            out_avals=tuple(out_avals),
            in_names=tuple(
                in_names + out_names + ([partition_name] if partition_name else [])
            ),
            out_names=tuple(out_names),
            lowering_input_output_aliases=(),
            sim_require_finite=True,
            sim_require_nnan=True,
            nc=nc,
        )
        return tuple(outs)

    devices = jax.devices()[:N_CORES]
    mesh = Mesh(np.asarray(devices), ("core",))
    spec = PartitionSpec("core")
    nshard = NamedSharding(mesh, spec)
    fn = jax.jit(
        shard_map(
            _body,
            mesh=mesh,
            in_specs=(spec,) * (n_params + len(out_names)),
            out_specs=(spec,) * len(out_names),
            check_rep=False,
        ),
        keep_unused=True,
    )
    per_core = {
        "x": [xp[i * N_PER_CORE : (i + 1) * N_PER_CORE] for i in range(N_CORES)],
        "wt": [wt] * N_CORES,
    }
    for name, arr in (extra or {}).items():
        per_core[name] = [arr] * N_CORES
    args = [np.concatenate(per_core[name], axis=0) for name in in_names] + [
        np.zeros((N_CORES * z.shape[0], *z.shape[1:]), z.dtype) for z in zero_outs
    ]
    dev_args = [jax.device_put(a, nshard) for a in args]
    idx = {name: i for i, name in enumerate(in_names)}
    return fn, dev_args, idx, nshard


def time_kernel(inputs, rep_big=1025, pairs=8, **build_kw):
    """Isolate on-device kernel time with ONE executable whose For_i trip
    count is a runtime input: wall(rep_big) - wall(1), / (rep_big - 1).
    Alternates the two trip counts to cancel slow drift."""
    import time

    import jax

    nc = _build_nc(dyn_rep=True, **build_kw)
    fn, dev_args, idx, nshard = _timed_runner(
        nc, inputs, extra={"rep": np.array([[1]], np.int32)}
    )
    ri = idx["rep"]

    def arg_set(k):
        a = list(dev_args)
        a[ri] = jax.device_put(
            np.concatenate([np.array([[k]], np.int32)] * N_CORES, axis=0), nshard
        )
        return a

    a1, ab = arg_set(1), arg_set(rep_big)
    for a in (a1, ab):  # compile + warm both trip counts
        jax.block_until_ready(fn(*a))

    t1s, tbs = [], []
    for _ in range(pairs):
        t0 = time.perf_counter()
        jax.block_until_ready(fn(*a1))
        t1s.append(time.perf_counter() - t0)
        t0 = time.perf_counter()
        jax.block_until_ready(fn(*ab))
        tbs.append(time.perf_counter() - t0)
    t1, tb = min(t1s), min(tbs)
    per_exec = (tb - t1) / (rep_big - 1)
    return per_exec * 1e9, {"t1": t1s, "tbig": tbs, "rep_big": rep_big}
